# revision 11
# baseline (speedup 1.0000x reference)
"""CPC spatial BCE loss kernel for 8 TRN2 NeuronCores.

Computation: loss = BCE(sigmoid((V1.reshape(N,D) @ V2.reshape(N,D).T) / D), eye(N))
with N=256, D=64*64*64=262144.

Strategy (memory-regime): shard the contraction dim D across the 8 cores
(64 MB of fp32 input per core -- the minimal possible HBM traffic). Each
core computes a partial Gram matrix [256, 256] over its 32768-wide slice
of D via bf16 TensorE matmuls accumulated in fp32 PSUM. The host lays out
each core's chunk d-major and pre-tiled to the exact SBUF tile layout, so
every chunk DMA is one fully-contiguous read with the contraction dim
landing on SBUF partitions -- no on-device transposes.

Device pipeline per chunk: fp32 loads stream on the two HWDGE rings
(SP ring for f1, ACT ring for f2 -- together they saturate the per-core
HBM share of ~358 GB/s); DVE casts fp32->bf16 (fp32 matmul on trn2 costs
4x: two HI/LO passes at half streaming rate, so bf16 compute is required
to stay under the DMA); TensorE runs the 32 matmuls per chunk into 2
PSUM accumulators. Chunk sizes are small at the head (fast pipeline fill)
and tail (short drain after the last DMA).

The partial Gram matrices are summed on the host (the unshard step for a
sum-sharded value) and the final sigmoid+BCE over 256x256 values is a
negligible epilogue done in numpy.
"""

import numpy as np

N = 256
D = 64 * 64 * 64  # 262144
NCORES = 8
DLOC = D // NCORES  # 32768
P = 128  # SBUF partitions
MB = 16  # max d-blocks of 128 per DMA chunk

_built = {}
_last_results = None  # test harness reads profiling info from here


def _sched(dloc=DLOC, mb=MB):
    """Chunk schedule in 128-d blocks: small head (fill) / tail (drain)."""
    nblocks = dloc // P
    if nblocks <= 4 * mb:
        sched = []
        rem = nblocks
        while rem > 0:
            s = min(4, rem)
            sched.append(s)
            rem -= s
        return sched
    sched = [4, 4, 8]
    while sum(sched) + mb <= nblocks - 8:
        sched.append(mb)
    rem = nblocks - sum(sched)
    while rem > 0:
        s = min(4, rem)
        sched.append(s)
        rem -= s
    assert sum(sched) == nblocks, (sched, nblocks)
    return sched


def _build(dloc=DLOC, mb=MB, compute_dtype="bf16", bufs=3):
    """Build + bacc-compile the per-core Bass kernel.

    Per-core inputs f1t, f2t: [dloc, N] fp32, host pre-tiled so chunk c
    (covering blocks [b0, b0+cmb) of 128 d-values) occupies rows
    [b0*P, (b0+cmb)*P) with row r = b0*P + p*cmb + nb holding
    f.T[d, :] for d = core_off + (b0+nb)*P + p  -- i.e. each chunk DMA
    is one contiguous read mapping partition p <- d within block.
    Output: out[i, j] = sum_d f1t[d, i] * f2t[d, j]   (partial Gram)
    """
    import concourse.mybir as mybir
    from concourse import bacc
    from concourse.bass import MemorySpace
    from concourse.tile import TileContext

    sched = _sched(dloc, mb)
    nblocks = dloc // P

    cdt = mybir.dt.bfloat16 if compute_dtype == "bf16" else mybir.dt.float32

    nc = bacc.Bacc("TRN2", target_bir_lowering=False, debug=False,
                   num_devices=NCORES)
    f1t = nc.dram_tensor("f1t", (dloc, N), mybir.dt.float32,
                         kind="ExternalInput")
    f2t = nc.dram_tensor("f2t", (dloc, N), mybir.dt.float32,
                         kind="ExternalInput")
    out = nc.dram_tensor("out", (N, N), mybir.dt.float32,
                         kind="ExternalOutput")

    f1v = f1t.ap()
    f2v = f2t.ap()

    with TileContext(nc) as tc:
        with tc.tile_pool(name="psum", bufs=1, space=MemorySpace.PSUM) as psum_pool, \
             tc.tile_pool(name="sbuff", bufs=bufs + 1) as poolf, \
             tc.tile_pool(name="sbufb", bufs=bufs) as poolb, \
             tc.tile_pool(name="outp", bufs=1) as outpool:
            acc = [psum_pool.tile([P, N], mybir.dt.float32, tag=f"acc{ib}",
                                  name=f"acc{ib}")
                   for ib in range(2)]
            b0 = 0
            for c, cmb in enumerate(sched):
                rsl = slice(b0 * P, (b0 + cmb) * P)
                # fp32 loads: two HWDGE rings (SP + ACT) stream concurrently
                t1f = poolf.tile([P, mb, N], mybir.dt.float32, tag="t1f",
                                 name=f"t1f_{c}")[:, :cmb]
                t2f = poolf.tile([P, mb, N], mybir.dt.float32, tag="t2f",
                                 name=f"t2f_{c}")[:, :cmb]
                nc.sync.dma_start(
                    out=t1f, in_=f1v[rsl].rearrange("(p nb) i -> p nb i", p=P))
                nc.scalar.dma_start(
                    out=t2f, in_=f2v[rsl].rearrange("(p nb) i -> p nb i", p=P))
                if cdt == mybir.dt.float32:
                    t1, t2 = t1f, t2f
                else:
                    # cast fp32->bf16 on DVE only: SP/ACT stay pure DMA rings
                    # (casts on ACT block its ring's next DMA issue in the
                    # engine FIFO; GpSimd shares its SBUF port with DVE and
                    # just contends). DVE ~1.2ns/elem stays under the chunk
                    # DMA cadence.
                    t1 = poolb.tile([P, mb, N], cdt, tag="t1",
                                    name=f"t1_{c}")[:, :cmb]
                    t2 = poolb.tile([P, mb, N], cdt, tag="t2",
                                    name=f"t2_{c}")[:, :cmb]
                    nc.vector.tensor_copy(t1, t1f)
                    nc.vector.tensor_copy(t2, t2f)
                for nb in range(cmb):
                    gb = b0 + nb
                    for ib in range(2):
                        nc.tensor.matmul(
                            acc[ib],
                            t1[:, nb, ib * P:(ib + 1) * P],  # lhsT [128d, 128i]
                            t2[:, nb, :],                     # rhs  [128d, 256j]
                            start=(gb == 0),
                            stop=(gb == nblocks - 1),
                        )
                b0 += cmb
            for ib in range(2):
                o = outpool.tile([P, N], mybir.dt.float32, tag=f"o{ib}",
                                 name=f"o{ib}")
                nc.vector.tensor_copy(o, acc[ib])
                nc.sync.dma_start(out=out.ap()[ib * P:(ib + 1) * P, :], in_=o)

    nc.compile()
    return nc


def _get_nc():
    if "nc" not in _built:
        _built["nc"] = _build()
    return _built["nc"]


def _gram_partials(in_maps, trace=False):
    global _last_results
    from concourse.bass_utils import run_bass_kernel_spmd

    nc = _get_nc()
    res = run_bass_kernel_spmd(nc, in_maps, core_ids=list(range(NCORES)),
                               trace=trace)
    _last_results = res
    return [r["out"] for r in res.results]


def _tile_layout(f, k, dloc=DLOC, mb=MB):
    """Core k's d-chunk of f [N, D], pre-tiled per _sched to [dloc, N].

    Chunk c covering blocks [b0, b0+cmb): rows [b0*P+(p*cmb+nb)] hold
    f[:, k*dloc + (b0+nb)*P + p] so the chunk is one contiguous span in
    the exact [P, cmb, N] SBUF tile order.
    """
    x = f[:, k * dloc:(k + 1) * dloc]          # [N, dloc]
    outa = np.empty((dloc, N), dtype=np.float32)
    b0 = 0
    for cmb in _sched(dloc, mb):
        sl = x[:, b0 * P:(b0 + cmb) * P]       # [N, cmb*P]
        t = sl.reshape(N, cmb, P).transpose(2, 1, 0)  # [P, cmb, N]
        outa[b0 * P:(b0 + cmb) * P] = t.reshape(cmb * P, N)
        b0 += cmb
    return outa


def kernel(V1, V2):
    V1 = np.asarray(V1, dtype=np.float32)
    V2 = np.asarray(V2, dtype=np.float32)
    f1 = V1.reshape(N, D)
    f2 = V2.reshape(N, D)

    in_maps = [
        {"f1t": _tile_layout(f1, k), "f2t": _tile_layout(f2, k)}
        for k in range(NCORES)
    ]
    partials = _gram_partials(in_maps)

    Z = np.zeros((N, N), dtype=np.float64)
    for pmat in partials:
        Z += pmat
    Z /= D

    eps = 1e-12
    p = 1.0 / (1.0 + np.exp(-Z))
    p = np.clip(p, eps, 1.0 - eps)
    lab = np.eye(N, dtype=np.float64)
    loss = -np.mean(lab * np.log(p) + (1.0 - lab) * np.log1p(-p))
    return np.array(loss, dtype=np.float32)


def _selftest_sim():
    """Scaled-down correctness check in CoreSim (no hardware)."""
    from concourse.bass_interp import CoreSim

    dloc, mb = 2048, 4
    nc = _build(dloc=dloc, mb=mb)
    rng = np.random.default_rng(0)
    a = rng.standard_normal((N, dloc)).astype(np.float32)  # [N, dloc] like f1
    b = rng.standard_normal((N, dloc)).astype(np.float32)

    sim = CoreSim(nc)
    sim.tensor("f1t")[:] = _tile_layout(a, 0, dloc=dloc, mb=mb)
    sim.tensor("f2t")[:] = _tile_layout(b, 0, dloc=dloc, mb=mb)
    sim.simulate()
    got = np.array(sim.tensor("out"))
    want = a.astype(np.float64) @ b.astype(np.float64).T
    err = np.abs(got - want).max() / np.abs(want).max()
    print("selftest rel err:", err)
    assert err < 2e-2, err
    print("SELFTEST PASSED")


if __name__ == "__main__":
    _selftest_sim()
